# revision 1
# baseline (speedup 1.0000x reference)
# GAT decoder kernel for Trainium2 (8 NeuronCores, SPMD + AllGather).
#
# Sharding: nodes (by destination) split across 8 cores. Per core:
#   1. MLP z->x1->x2->xpx with feature-major fp32r matmuls. Wg is host-extended
#      with folded att_src/att_dst columns so per-node attention scalars fall
#      out of the same matmul.
#   2. xpx transposed to node-major fp16 rows [xp(256)|a_src(2)|a_dst(2)|pad]
#      (768B each), AllGather -> full gather table in DRAM.
#   3. Edge phase: host pre-sorts edges by dst into 32-row windows with
#      core-uniform chunk caps (one shared instruction stream). Per 128-edge
#      chunk: dma_gather pulls xp rows into SBUF; DVE/ACT build the sparse
#      S matrix (mask * exp(leaky_relu(a_src+a_dst))); PE accumulates
#      S^T @ G (messages) and S^T @ 1 (denominators) into PSUM per
#      128-dst super-chunk.
#   4. Self-loop contribution + normalization, transpose, final @ W3.
import os

import numpy as np

LATENT, HID, OUT, HEADS = 512, 256, 128, 2
NEG = 0.2
N_CORES = 8
W = 32           # dst rows per window
GMAX = 8         # chunks per dma_gather call (<=1024 idx HW limit)
ROW = 384        # fp16 elements per gather-table row (768 bytes)
C = HEADS * OUT  # 256 message channels
LAST_EXEC_NS = None
LAST_RESULT = None


def _plan(src, dst, NB, n_sc, sc_rows):
    """Sort edges by dst, carve into W-row windows / 128-edge chunks with caps
    shared by all cores (identical instruction stream)."""
    win_index = []
    for s in range(n_sc):
        for wi in range(int(np.ceil(sc_rows[s] / W))):
            win_index.append((s, wi))
    n_win = len(win_index)
    win_id = {sw: i for i, sw in enumerate(win_index)}

    per_core = []
    counts = np.zeros((N_CORES, n_win), np.int64)
    for c in range(N_CORES):
        sel = (dst >= c * NB) & (dst < (c + 1) * NB)
        d = dst[sel] - c * NB
        s_ = src[sel]
        o = np.argsort(d, kind="stable")
        d, s_ = d[o], s_[o]
        sc = d // 128
        wi = (d % 128) // W
        key = sc * 0
        if len(d):
            key = np.array([win_id[(int(a), int(b))] for a, b in zip(sc, wi)], np.int64)
        cnt = np.bincount(key, minlength=n_win) if len(d) else np.zeros(n_win, np.int64)
        counts[c] = cnt
        per_core.append((d, s_, key, cnt))

    caps = np.maximum(1, np.ceil(counts / 128.0).astype(np.int64).max(axis=0))
    chunk_base = np.concatenate([[0], np.cumsum(caps)])
    NCH = int(chunk_base[-1])
    chunk_sw = []
    for i, sw in enumerate(win_index):
        chunk_sw += [sw] * int(caps[i])

    gidx_all, mask_all = [], []
    for c in range(N_CORES):
        d, s_, key, cnt = per_core[c]
        start = np.concatenate([[0], np.cumsum(cnt)])
        gidx = np.zeros(NCH * 128, np.int16)
        mask = np.zeros((128, NCH, W), np.float16)
        if len(d):
            r = np.arange(len(d)) - start[key]
            k = (chunk_base[key] + r // 128).astype(np.int64)
            p = (r % 128).astype(np.int64)
            gidx[k * 128 + p] = s_.astype(np.int16)
            mask[p, k, d % W] = 1.0
        i = np.arange(NCH * 128)
        gw = np.zeros((128, NCH * 8), np.int16)
        gw[(i % 16)[None, :] + 16 * np.arange(8)[:, None], (i // 16)[None, :]] = gidx[None, :]
        gidx_all.append(gw)
        mask_all.append(mask)
    return gidx_all, mask_all, chunk_sw, NCH


def _build(N, NB, n_sc, sc_rows, chunk_sw, NCH):
    import concourse.bass as bass
    import concourse.bacc as bacc
    import concourse.tile as tile
    import concourse.mybir as mybir
    from concourse.tile import add_dep_helper
    from concourse.masks import make_identity

    f16, f32, f32r, i16 = (mybir.dt.float16, mybir.dt.float32,
                           mybir.dt.float32r, mybir.dt.int16)
    NPAD = N + 96
    nc = bacc.Bacc("TRN2", num_devices=N_CORES)

    z_h = nc.dram_tensor("z", [NB, LATENT], f32, kind="ExternalInput")
    w1_h = nc.dram_tensor("w1", [LATENT, HID], f32, kind="ExternalInput")
    b1_h = nc.dram_tensor("b1", [HID], f32, kind="ExternalInput")
    w2_h = nc.dram_tensor("w2", [HID, 2 * C], f32, kind="ExternalInput")
    b2_h = nc.dram_tensor("b2", [2 * C], f32, kind="ExternalInput")
    wg_h = nc.dram_tensor("wg", [2 * C, 264], f32, kind="ExternalInput")
    bg_h = nc.dram_tensor("bg", [264], f32, kind="ExternalInput")
    w3_h = nc.dram_tensor("w3", [C, LATENT], f32, kind="ExternalInput")
    b3_h = nc.dram_tensor("b3", [LATENT], f32, kind="ExternalInput")
    gidx_h = nc.dram_tensor("gidx", [128, NCH * 8], i16, kind="ExternalInput")
    mask_h = nc.dram_tensor("mask", [128, NCH, W], f16, kind="ExternalInput")
    y_h = nc.dram_tensor("y", [NB, LATENT], f32, kind="ExternalOutput")

    shard = nc.dram_tensor("shard", [NB, ROW], f16)
    tab = nc.dram_tensor("tab", [NPAD, ROW], f16, addr_space="Shared")
    # local node-major rows for self-loop/normalization reads (avoids
    # core-dependent offsets into the gathered table)
    loc = nc.dram_tensor("loc", [n_sc * 128, ROW], f16)

    sc_first = [None] * n_sc
    sc_n = [0] * n_sc
    for k, (s, wi) in enumerate(chunk_sw):
        if sc_first[s] is None:
            sc_first[s] = k
        sc_n[s] += 1
    NKMAX = max(sc_n)
    n_nt = (NB + 127) // 128
    assert n_nt == n_sc

    with tile.TileContext(nc) as tc:
        const = tc.alloc_tile_pool(name="const", bufs=1)

        # ---- weights ----
        w1t = const.tile([128, 4, HID], f32r)
        nc.sync.dma_start(out=w1t, in_=w1_h.ap().rearrange("(kc k) o -> k kc o", k=128).bitcast(f32r))
        w2t = const.tile([128, 2, 2 * C], f32r)
        nc.sync.dma_start(out=w2t, in_=w2_h.ap().rearrange("(kc k) o -> k kc o", k=128).bitcast(f32r))
        wgt = const.tile([128, 4, 264], f32r)
        nc.sync.dma_start(out=wgt, in_=wg_h.ap().rearrange("(kc k) o -> k kc o", k=128).bitcast(f32r))
        w3t = const.tile([128, 2, LATENT], f16)
        nc.gpsimd.dma_start(out=w3t, in_=w3_h.ap().rearrange("(kc k) o -> k kc o", k=128))
        b1t = const.tile([128, 2], f32)
        nc.sync.dma_start(out=b1t, in_=b1_h.ap().rearrange("(oc p) -> p oc", p=128))
        b2t = const.tile([128, 4], f32)
        nc.sync.dma_start(out=b2t, in_=b2_h.ap().rearrange("(oc p) -> p oc", p=128))
        bgt = const.tile([128, 3], f32)
        nc.sync.dma_start(out=bgt[:, 0:2], in_=bg_h.ap()[0:256].rearrange("(oc p) -> p oc", p=128))
        nc.sync.dma_start(out=bgt[0:8, 2:3], in_=bg_h.ap()[256:264].rearrange("(oc p) -> p oc", p=8))
        b3rep = const.tile([128, LATENT], f32)
        nc.sync.dma_start(out=b3rep, in_=bass.AP(tensor=b3_h, offset=0, ap=[[0, 128], [1, LATENT]]))
        gidx_t = const.tile([128, NCH * 8], i16)
        nc.sync.dma_start(out=gidx_t, in_=gidx_h.ap())
        ones_t = const.tile([128, 1], f16)
        nc.vector.memset(ones_t, 1.0)
        gmark_t = const.tile([128, 2], f32)
        ident = const.tile([128, 128], f32)
        make_identity(nc, ident)
        identh = const.tile([128, 128], f16)
        nc.vector.tensor_copy(out=identh, in_=ident)

        tps = tc.alloc_tile_pool(name="tps", bufs=4, space="PSUM")
        # ---- MLP phase ----
        mlp = tc.alloc_tile_pool(name="mlp", bufs=1)
        mps = tc.alloc_tile_pool(name="mps", bufs=2, space="PSUM")

        zT = mlp.tile([128, 4, NB], f32r)
        x1T = mlp.tile([128, 2, NB], f32r)
        x2T = mlp.tile([128, 4, NB], f32r)
        xpxT = mlp.tile([128, 3, NB], f32)

        zstage = tc.alloc_tile_pool(name="zstage", bufs=3)
        for nt in range(n_nt):
            r0, nr = nt * 128, min(128, NB - nt * 128)
            zn = zstage.tile([128, LATENT], f32, name="zn")
            nc.sync.dma_start(out=zn[0:nr, :], in_=z_h.ap()[r0:r0 + nr, :])
            for kc in range(4):
                pt = tps.tile([128, 128], f32, name="zt", tag="tp")
                nc.tensor.transpose(pt[:, 0:nr], zn[0:nr, kc * 128:(kc + 1) * 128], ident[0:nr, 0:nr])
                nc.scalar.copy(zT[:, kc, r0:r0 + nr], pt[:, 0:nr])

        NT = 500
        n_mt = (NB + NT - 1) // NT
        for mt in range(n_mt):
            c0, cn = mt * NT, min(NT, NB - mt * NT)
            for oc in range(2):
                ps = mps.tile([128, NT], f32, name="mm1", tag="mm")
                for kc in range(4):
                    nc.tensor.matmul(ps[:, 0:cn], lhsT=w1t[:, kc, oc * 128:(oc + 1) * 128],
                                     rhs=zT[:, kc, c0:c0 + cn], start=(kc == 0), stop=(kc == 3))
                nc.scalar.activation(out=x1T[:, oc, c0:c0 + cn], in_=ps[:, 0:cn],
                                     func=mybir.ActivationFunctionType.Relu, bias=b1t[:, oc:oc + 1])
        for mt in range(n_mt):
            c0, cn = mt * NT, min(NT, NB - mt * NT)
            for oc in range(4):
                ps = mps.tile([128, NT], f32, name="mm2", tag="mm")
                for kc in range(2):
                    nc.tensor.matmul(ps[:, 0:cn], lhsT=w2t[:, kc, oc * 128:(oc + 1) * 128],
                                     rhs=x1T[:, kc, c0:c0 + cn], start=(kc == 0), stop=(kc == 1))
                nc.scalar.activation(out=x2T[:, oc, c0:c0 + cn], in_=ps[:, 0:cn],
                                     func=mybir.ActivationFunctionType.Relu, bias=b2t[:, oc:oc + 1])
        for mt in range(n_mt):
            c0, cn = mt * NT, min(NT, NB - mt * NT)
            for oc in range(3):
                ow = 128 if oc < 2 else 8
                ps = mps.tile([128, NT], f32, name="mm3", tag="mm")
                for kc in range(4):
                    nc.tensor.matmul(ps[0:ow, 0:cn], lhsT=wgt[:, kc, oc * 128:oc * 128 + ow],
                                     rhs=x2T[:, kc, c0:c0 + cn], start=(kc == 0), stop=(kc == 3))
                nc.scalar.activation(out=xpxT[0:ow, oc, c0:c0 + cn], in_=ps[0:ow, 0:cn],
                                     func=mybir.ActivationFunctionType.Identity, bias=bgt[0:ow, oc:oc + 1])

        # ---- node-major rows, shard write ----
        xpn_pool = tc.alloc_tile_pool(name="xpnp", bufs=3)
        shard_dmas = []
        for nt in range(n_nt):
            r0, nr = nt * 128, min(128, NB - nt * 128)
            xpn = xpn_pool.tile([128, ROW], f16, name="xpn")
            for g in range(2):
                pt = tps.tile([128, 128], f32, name="xpt", tag="tp")
                nc.tensor.transpose(pt[0:nr, :], xpxT[:, g, r0:r0 + nr], ident)
                nc.scalar.copy(xpn[0:nr, g * 128:(g + 1) * 128], pt[0:nr, :])
            pa = tps.tile([128, 8], f32, name="xpa", tag="tp")
            nc.tensor.transpose(pa[0:nr, :], xpxT[0:8, 2, r0:r0 + nr], ident[0:8, 0:8])
            nc.scalar.copy(xpn[0:nr, 256:264], pa[0:nr, :])
            d1 = nc.sync.dma_start(out=shard.ap()[r0:r0 + nr, :], in_=xpn[0:nr, :])
            d2 = nc.sync.dma_start(out=loc.ap()[r0:r0 + nr, :], in_=xpn[0:nr, :])
            shard_dmas += [d1, d2]

        cc = nc.gpsimd.collective_compute(
            "AllGather", mybir.AluOpType.bypass,
            replica_groups=[list(range(N_CORES))],
            ins=[shard.ap()], outs=[tab.ap()[0:N, :]])
        for d in shard_dmas:
            add_dep_helper(cc.ins, d.ins, sync=True, reason="allgather waits shard writes")

        xpn_pool.release()
        zstage.release()
        mps.release()
        mlp.release()

        # ---- edge phase ----
        work = tc.alloc_tile_pool(name="work", bufs=2)
        aps_pool = tc.alloc_tile_pool(name="aps", bufs=2, space="PSUM")
        yps_pool = tc.alloc_tile_pool(name="yps", bufs=2, space="PSUM")

        gsem = nc.alloc_semaphore("gsem")
        psem = nc.alloc_semaphore("psem")
        n_prep = 0
        for sc in range(n_sc):
            k0, nk = sc_first[sc], sc_n[sc]
            nrows = sc_rows[sc]
            # gather G rows for this super-chunk
            G = work.tile([128, NKMAX, ROW], f16, name="G")
            for c0 in range(0, nk, GMAX):
                cn = min(GMAX, nk - c0)
                n_prep += 1
                with tc.tile_critical():
                    gi = nc.gpsimd.dma_gather(
                        G[:, c0:c0 + cn, :], tab.ap(),
                        gidx_t[:, (k0 + c0) * 8:(k0 + c0 + cn) * 8],
                        num_idxs=cn * 128, num_idxs_reg=cn * 128,
                        elem_size=ROW, prepare_only=True, sem=gsem).then_inc(psem, 1)
                    nc.gpsimd.wait_ge(psem, n_prep)
                    nc.gpsimd.trigger_dma(count=1)
                add_dep_helper(gi.ins, cc.ins, sync=False, reason="gather after allgather")
            with tc.tile_critical():
                nc.gpsimd.wait_ge(gsem, 16 * n_prep)
            gmark = nc.gpsimd.memset(gmark_t[0:1, sc % 2:sc % 2 + 1], 0)

            def dep_g(inst):
                add_dep_helper(inst.ins, gmark.ins, sync=True, reason="G landed")

            # mask slice + adst replicate
            mask_t = work.tile([128, NKMAX, W], f16, name="maskt")
            nc.sync.dma_start(out=mask_t[:, 0:nk, :], in_=mask_h.ap()[:, k0:k0 + nk, :])
            adst_rep = work.tile([128, 128, 2], f16, name="adrep")
            da = nc.sync.dma_start(out=adst_rep, in_=bass.AP(
                tensor=loc, offset=(sc * 128) * ROW + 258, ap=[[0, 128], [ROW, 128], [1, 2]]))

            # S build per window slab
            Sh = [work.tile([128, NKMAX, W], f16, name=f"S{h}") for h in range(2)]
            lg = work.tile([128, NKMAX, W], f16, name="lg")
            lg2 = work.tile([128, NKMAX, W], f16, name="lg2")
            win_chunks = {}
            for k in range(k0, k0 + nk):
                win_chunks.setdefault(chunk_sw[k][1], []).append(k)
            for h in range(2):
                for wi, wks in sorted(win_chunks.items()):
                    kk0, knn = wks[0] - k0, len(wks)
                    lgv = lg[:, kk0:kk0 + knn, :]
                    asrc_b = bass.AP(tensor=G.tensor, offset=G.offset + kk0 * ROW + 256 + h,
                                     ap=[list(G.ap[0]), [ROW, knn], [0, W]])
                    adst_b = bass.AP(tensor=adst_rep.tensor,
                                     offset=adst_rep.offset + (wi * W) * 2 + h,
                                     ap=[list(adst_rep.ap[0]), [0, knn], [2, W]])
                    ad = nc.vector.tensor_tensor(out=lgv, in0=asrc_b, in1=adst_b, op=mybir.AluOpType.add)
                    dep_g(ad)
                    lg2v = lg2[:, kk0:kk0 + knn, :]
                    nc.vector.tensor_scalar(out=lg2v, in0=lgv, scalar1=NEG, scalar2=None,
                                            op0=mybir.AluOpType.mult)
                    nc.vector.tensor_tensor(out=lgv, in0=lgv, in1=lg2v, op=mybir.AluOpType.max)
                    nc.scalar.activation(out=lgv, in_=lgv, func=mybir.ActivationFunctionType.Exp)
                    nc.vector.tensor_tensor(out=Sh[h][:, kk0:kk0 + knn, :],
                                            in0=mask_t[:, kk0:kk0 + knn, :], in1=lgv,
                                            op=mybir.AluOpType.mult)

            # aggregation matmuls
            ps = aps_pool.tile([128, C + 4], f32, name="agg")
            nc.vector.memset(ps, 0.0)
            for k in range(k0, k0 + nk):
                wi = chunk_sw[k][1]
                w0 = wi * W
                kk = k - k0
                for h in range(2):
                    m1 = nc.tensor.matmul(ps[w0:w0 + W, h * OUT:(h + 1) * OUT],
                                          lhsT=Sh[h][:, kk, :], rhs=G[:, kk, h * OUT:(h + 1) * OUT],
                                          start=False, stop=False, tile_position=(0, w0),
                                          skip_group_check=True)
                    dep_g(m1)
                    nc.tensor.matmul(ps[w0:w0 + W, C + 2 * h:C + 2 * h + 1],
                                     lhsT=Sh[h][:, kk, :], rhs=ones_t,
                                     start=False, stop=False, tile_position=(0, w0),
                                     skip_group_check=True)

            # ---- self loops + normalize ----
            aslab = work.tile([128, 4], f16, name="aslab")
            nc.sync.dma_start(out=aslab, in_=bass.AP(
                tensor=loc, offset=(sc * 128) * ROW + 256, ap=[[ROW, 128], [1, 4]]))
            xploc = work.tile([128, C], f16, name="xploc")
            nc.sync.dma_start(out=xploc, in_=bass.AP(
                tensor=loc, offset=(sc * 128) * ROW, ap=[[ROW, 128], [1, C]]))
            lsf = work.tile([128, 2], f32, name="lsf")
            nc.vector.tensor_tensor(out=lsf, in0=aslab[:, 0:2], in1=aslab[:, 2:4], op=mybir.AluOpType.add)
            lsf2 = work.tile([128, 2], f32, name="lsf2")
            nc.vector.tensor_scalar(out=lsf2, in0=lsf, scalar1=NEG, scalar2=None, op0=mybir.AluOpType.mult)
            nc.vector.tensor_tensor(out=lsf, in0=lsf, in1=lsf2, op=mybir.AluOpType.max)
            wself = work.tile([128, 2], f32, name="wself")
            nc.scalar.activation(out=wself, in_=lsf, func=mybir.ActivationFunctionType.Exp)

            gat = work.tile([128, C], f32, name="gat")
            den = work.tile([128, 2], f32, name="den")
            for h in range(2):
                nc.vector.tensor_scalar(out=gat[:, h * OUT:(h + 1) * OUT],
                                        in0=xploc[:, h * OUT:(h + 1) * OUT],
                                        scalar1=wself[:, h:h + 1], scalar2=None,
                                        op0=mybir.AluOpType.mult)
                nc.vector.tensor_tensor(out=gat[:, h * OUT:(h + 1) * OUT],
                                        in0=gat[:, h * OUT:(h + 1) * OUT],
                                        in1=ps[:, h * OUT:(h + 1) * OUT], op=mybir.AluOpType.add)
                nc.vector.tensor_tensor(out=den[:, h:h + 1], in0=wself[:, h:h + 1],
                                        in1=ps[:, C + 2 * h:C + 2 * h + 1], op=mybir.AluOpType.add)
            rden = work.tile([128, 2], f32, name="rden")
            nc.vector.reciprocal(out=rden, in_=den)
            gatn = work.tile([128, C], f16, name="gatn")
            for h in range(2):
                nc.vector.tensor_scalar(out=gatn[:, h * OUT:(h + 1) * OUT],
                                        in0=gat[:, h * OUT:(h + 1) * OUT],
                                        scalar1=rden[:, h:h + 1], scalar2=None,
                                        op0=mybir.AluOpType.mult)

            # ---- transpose + final matmul ----
            gatT = work.tile([128, 2, 128], f16, name="gatT")
            for g in range(2):
                ptt = tps.tile([128, 128], f16, name="gtt", tag="tp")
                nc.tensor.transpose(ptt, gatn[:, g * 128:(g + 1) * 128], identh)
                nc.scalar.copy(gatT[:, g, :], ptt)
            yps = yps_pool.tile([128, LATENT], f32, name="yps")
            for g in range(2):
                nc.tensor.matmul(yps, lhsT=gatT[:, g, :],
                                 rhs=w3t[:, g, :], start=(g == 0), stop=(g == 1))
            ysb = work.tile([128, LATENT], f32, name="ysb")
            nc.vector.tensor_tensor(out=ysb, in0=yps, in1=b3rep, op=mybir.AluOpType.add)
            nc.sync.dma_start(out=y_h.ap()[sc * 128:sc * 128 + nrows, :], in_=ysb[0:nrows, :])

        yps_pool.release()
        aps_pool.release()
        work.release()
        tps.release()
        const.release()

    nc.compile()
    return nc


def _prepare(inputs):
    z = np.asarray(inputs["z"], np.float32)
    ei = np.asarray(inputs["edge_index"], np.int64)
    W1 = np.asarray(inputs["W1"], np.float32)
    b1 = np.asarray(inputs["b1"], np.float32)
    W2 = np.asarray(inputs["W2"], np.float32)
    b2 = np.asarray(inputs["b2"], np.float32)
    Wg = np.asarray(inputs["Wg"], np.float32)
    att_src = np.asarray(inputs["att_src"], np.float32)
    att_dst = np.asarray(inputs["att_dst"], np.float32)
    bias_g = np.asarray(inputs["bias_g"], np.float32)
    W3 = np.asarray(inputs["W3"], np.float32)
    b3 = np.asarray(inputs["b3"], np.float32)

    N = z.shape[0]
    NB = N // N_CORES
    n_sc = (NB + 127) // 128
    sc_rows = [min(128, NB - s * 128) for s in range(n_sc)]

    # fold attention vectors into Wg extension columns
    KG = Wg.shape[0]
    vs = np.zeros((KG, 2), np.float32)
    vd = np.zeros((KG, 2), np.float32)
    for h in range(HEADS):
        vs[:, h] = Wg[:, h * OUT:(h + 1) * OUT] @ att_src[h]
        vd[:, h] = Wg[:, h * OUT:(h + 1) * OUT] @ att_dst[h]
    wg_ext = np.concatenate([Wg, vs, vd, np.zeros((KG, 4), np.float32)], axis=1)
    ab_src = np.array([bias_g[h * OUT:(h + 1) * OUT] @ att_src[h] for h in range(HEADS)], np.float32)
    ab_dst = np.array([bias_g[h * OUT:(h + 1) * OUT] @ att_dst[h] for h in range(HEADS)], np.float32)
    bg_ext = np.concatenate([bias_g, ab_src, ab_dst, np.zeros(4, np.float32)])

    gidx_all, mask_all, chunk_sw, NCH = _plan(ei[0], ei[1], NB, n_sc, sc_rows)

    nc = _build(N, NB, n_sc, sc_rows, chunk_sw, NCH)

    in_maps = []
    for c in range(N_CORES):
        in_maps.append({
            "z": z[c * NB:(c + 1) * NB],
            "w1": W1, "b1": b1, "w2": W2, "b2": b2,
            "wg": wg_ext, "bg": bg_ext, "w3": W3, "b3": b3,
            "gidx": gidx_all[c], "mask": mask_all[c],
        })
    return nc, in_maps


def kernel(**inputs):
    import sys
    if '/opt/trn_rl_repo' not in sys.path:
        sys.path.insert(0, '/opt/trn_rl_repo')
    from concourse.bass_utils import run_bass_kernel_spmd

    nc, in_maps = _prepare(inputs)
    res = run_bass_kernel_spmd(nc, in_maps, list(range(N_CORES)))
    global LAST_RESULT
    LAST_RESULT = res
    y = np.concatenate([res.results[c]["y"] for c in range(N_CORES)], axis=0)
    return y.astype(np.float32)

